# revision 22
# baseline (speedup 1.0000x reference)
"""Multihead self-attention (T=1024, B=4, E=1024, H=16) on 8 TRN2 NeuronCores.

Sharding: head-parallel. Core i owns heads {2i, 2i+1} == E-rows [128i, 128i+128)
of Wq/Wk/Wv, and all 4 batches. No cross-core communication.

Per-core dataflow (all "transposed" layouts, d on partitions):
  qT/kT/vT [128, B*T] = W_slice @ query.T   (PE, float32r, K=E in 8 chunks)
  per (b, head, t-chunk of 512):
    scoresT [s=128, t<=512] = kT_tile.T-free matmul; causal tiles above the
    diagonal are skipped entirely, diagonal tiles are column-sliced.
    probs = Exp(scoresT + causal_mask + padding_bias)  (ACT, padding as
    per-partition bias; both heads share one [128, 1024] ACT op)
    outT[65, 512] += va_tile[128, 65].T @ probs  where va has a ones column,
    so row 64 accumulates the softmax denominator.
  normalize: DMA-broadcast denominator row, DVE reciprocal + multiply.
Host gathers [128, B, T] per-core outputs -> [T, B, E].
"""

import numpy as np

T, B, E, H = 1024, 4, 1024, 16
D = 64  # head dim
NCORES = 8
HPC = H // NCORES  # heads per core = 2
DS = HPC * D  # per-core E-slice = 128
TB = T * B
NEG = -1.0e30
SCALE = D**-0.5

_COMPILED = {}


def _build_program():
    import concourse.bacc as bacc
    import concourse.mybir as mybir
    import concourse.tile as tile
    from concourse.masks import make_identity

    f32 = mybir.dt.float32
    f32r = mybir.dt.float32r
    AF = mybir.ActivationFunctionType
    ALU = mybir.AluOpType

    nc = bacc.Bacc("TRN2", target_bir_lowering=False, debug=False,
                   num_devices=NCORES)

    qt = nc.dram_tensor("qt", [E, B, T], f32r, kind="ExternalInput").ap()
    wq = nc.dram_tensor("wq", [E, DS], f32r, kind="ExternalInput").ap()
    wk = nc.dram_tensor("wk", [E, DS], f32r, kind="ExternalInput").ap()
    wv = nc.dram_tensor("wv", [E, DS], f32r, kind="ExternalInput").ap()
    bq = nc.dram_tensor("bq", [DS, 1], f32, kind="ExternalInput").ap()
    bk = nc.dram_tensor("bk", [DS, 1], f32, kind="ExternalInput").ap()
    bv = nc.dram_tensor("bv", [DS, 1], f32, kind="ExternalInput").ap()
    cm = nc.dram_tensor("cm", [4, 128, 512], f32, kind="ExternalInput").ap()
    pad = nc.dram_tensor("pad", [128, B * 8], f32, kind="ExternalInput").ap()
    onesd = nc.dram_tensor("ones", [128, 16], f32r, kind="ExternalInput").ap()
    out = nc.dram_tensor("out", [B, T, DS], f32, kind="ExternalOutput").ap()

    NJ = TB // 512  # 8 token chunks of 512; chunk j covers (b=j//2, half=j%2)

    with tile.TileContext(nc) as tc:
        with (
            tc.tile_pool(name="consts", bufs=1) as consts,
            tc.tile_pool(name="qkv", bufs=NJ) as qkv,
            tc.tile_pool(name="va", bufs=B) as vap,
            tc.tile_pool(name="probs", bufs=4) as probsp,
            tc.tile_pool(name="outsb", bufs=3) as outsb,
            tc.tile_pool(name="bcast", bufs=3) as bcastp,
        ):
            w_sb = {}
            b_sb = {}
            wdrs = {"q": wq, "k": wk, "v": wv}
            for nm in ("q", "k", "v"):
                w_sb[nm] = consts.tile([128, 8, DS], f32r, name=f"w{nm}s")
            # per-chunk weight loads so the first matmuls aren't gated on the
            # full 1.5 MB of weights
            for e in range(8):
                for nm in ("q", "k", "v"):
                    nc.scalar.dma_start(
                        w_sb[nm][:, e, :],
                        wdrs[nm].rearrange("(c p) m -> p c m", p=128)[:, e, :],
                    )
            for nm, bdr in (("q", bq), ("k", bk), ("v", bv)):
                bt = consts.tile([DS, 1], f32, name=f"b{nm}s")
                nc.scalar.dma_start(bt[:], bdr)
                b_sb[nm] = bt
            ident = consts.tile([128, 128], f32, name="ident")
            make_identity(nc, ident[:])
            cm_sb = consts.tile([128, 4, 512], f32, name="cms")
            nc.scalar.dma_start(cm_sb[:], cm.rearrange("d p f -> p d f"))
            pad_sb = consts.tile([128, B * 8], f32, name="pads")
            nc.scalar.dma_start(pad_sb[:], pad)

            # ---- Phase 1: QKV projection into transposed layout ----
            qkv_t = {"q": [], "k": [], "v": []}
            ph1 = tc.tile_pool(name="rhs", bufs=10)
            rhsp = ph1.__enter__()
            ph1b = tc.tile_pool(name="psA", bufs=2, space="PSUM")
            psA = ph1b.__enter__()
            for j in range(NJ):
                qu = []
                for e in range(8):
                    qtile = rhsp.tile([128, 512], f32r, tag="qu", name=f"qu{j}_{e}")
                    nc.sync.dma_start(
                        qtile[:],
                        qt.rearrange("e b t -> e (b t)")[
                            e * 128:(e + 1) * 128, j * 512:(j + 1) * 512
                        ],
                    )
                    qu.append(qtile)
                for nm in ("q", "k", "v"):
                    ps = psA.tile([128, 512], f32, tag="proj", name=f"ps{nm}{j}")
                    for e in range(8):
                        nc.tensor.matmul(
                            ps[:],
                            w_sb[nm][:, e, :],
                            qu[e][:],
                            start=(e == 0),
                            stop=(e == 7),
                        )
                    dst = qkv.tile([128, 512], f32r if nm != "v" else f32, tag=f"{nm}t", name=f"{nm}t{j}")
                    nc.vector.tensor_scalar(
                        dst[:], ps[:], b_sb[nm][:], None, op0=ALU.add,
                    )
                    qkv_t[nm].append(dst)

            ph1b.__exit__(None, None, None)
            ph1.__exit__(None, None, None)

            # ---- Phase 1.5: transpose vT -> va ([s, d] with ones column) ----
            va_t = []
            for b in range(B):
                va = vap.tile([128, 8, 2, 65], f32r, tag="va", name=f"va{b}")
                nc.scalar.dma_start(
                    va[:, :, :, 64:65],
                    onesd.rearrange("p (a b c) -> p a b c", a=8, b=2, c=1),
                )
                va_t.append(va)
            ph15 = tc.tile_pool(name="psTr", bufs=2, space="PSUM")
            psTr = ph15.__enter__()
            for b in range(B):
                for p in range(8):
                    j = 2 * b + p // 4
                    off = (p % 4) * 128
                    tp = psTr.tile([128, 128], f32, tag="tr", name=f"tr{b}_{p}")
                    nc.tensor.transpose(
                        tp[:], qkv_t["v"][j][:, off:off + 128], ident[:]
                    )
                    nc.vector.tensor_copy(
                        va_t[b][:, p, :, 0:64],
                        tp[:].rearrange("p (two sub) -> p two sub", two=2),
                    )
            ph15.__exit__(None, None, None)

            # ---- Phase 2: attention ----
            ph2 = tc.tile_pool(name="psS", bufs=2, space="PSUM")
            psS = ph2.__enter__()
            ph2b = tc.tile_pool(name="psO", bufs=3, space="PSUM")
            psO = ph2b.__enter__()
            ph2c = tc.tile_pool(name="psE", bufs=1, space="PSUM")
            psE = ph2c.__enter__()
            for b in range(B):
                for c in range(2):  # t-chunks of 512
                    j = 2 * b + c
                    po = []
                    for hl in range(HPC):
                        pot = psO.tile([65, 512], f32, tag="po",
                                       name=f"po{b}_{c}_{hl}")
                        po.append(pot)
                    ntile = 4 * (c + 1)
                    for p in range(ntile):
                        dlt = p - 4 * c
                        w0 = min(128 * dlt, 256) if dlt > 0 else 0
                        jk = 2 * b + p // 4
                        offk = (p % 4) * 128
                        ss = psS.tile([128, 2, 512], f32, tag="sc",
                                      name=f"sc{b}_{c}_{p}")
                        for hl in range(HPC):
                            nc.tensor.matmul(
                                ss[:, hl, w0:512],
                                qkv_t["k"][jk][hl * 64:(hl + 1) * 64,
                                               offk:offk + 128],
                                qkv_t["q"][j][hl * 64:(hl + 1) * 64,
                                              w0:512],
                                start=True,
                                stop=True,
                            )
                        if dlt >= 0:
                            nc.vector.tensor_tensor(
                                ss[:, :, w0:512],
                                ss[:, :, w0:512],
                                cm_sb[:, dlt, None, w0:512].to_broadcast(
                                    (128, 2, 512 - w0)
                                ),
                                ALU.add,
                            )
                        pr = probsp.tile([128, 2, 512], f32r, tag="pr",
                                         name=f"pr{b}_{c}_{p}")
                        nc.scalar.activation(
                            pr[:, :, w0:512],
                            ss[:, :, w0:512],
                            AF.Exp,
                            bias=pad_sb[:, b * 8 + p:b * 8 + p + 1],
                            scale=1.0,
                        )
                        for hl in range(HPC):
                            nc.tensor.matmul(
                                po[hl][:, w0:512],
                                va_t[b][:, p, hl, :],
                                pr[:, hl, w0:512],
                                start=(p == 0),
                                stop=(p == ntile - 1),
                            )
                    for hl in range(HPC):
                        # epilogue: copy po to SBUF (frees the accumulator),
                        # PE-transpose to [t, d] layout, per-partition
                        # reciprocal of the denominator column, scale, store.
                        pos = bcastp.tile([65, 512], f32, tag="pos",
                                          name=f"pos{b}_{c}_{hl}")
                        nc.scalar.activation(pos[:], po[hl][:], AF.Copy)
                        te = psE.tile([128, 4, 65], f32, tag="te",
                                      name=f"te{b}_{c}_{hl}")
                        for g in range(4):
                            nc.tensor.transpose(
                                te[:, g, :],
                                pos[:, g * 128:(g + 1) * 128],
                                ident[0:65, 0:65],
                            )
                        rcp = outsb.tile([128, 4, 1], f32, tag="rcp",
                                         name=f"rcp{b}_{c}_{hl}")
                        nc.vector.reciprocal(rcp[:], te[:, :, 64:65])
                        of = outsb.tile([128, 4, 64], f32, tag="of",
                                        name=f"of{b}_{c}_{hl}")
                        for g in range(4):
                            nc.vector.tensor_scalar(
                                of[:, g, :], te[:, g, 0:64], rcp[:, g, :],
                                None, op0=ALU.mult,
                            )
                        nc.sync.dma_start(
                            out[b, c * 512:(c + 1) * 512,
                                hl * 64:(hl + 1) * 64].rearrange(
                                    "(g tp) m -> tp g m", tp=128),
                            of[:],
                        )
            ph2c.__exit__(None, None, None)
            ph2b.__exit__(None, None, None)
            ph2.__exit__(None, None, None)

    nc.compile()
    return nc


def _get_program():
    if "nc" not in _COMPILED:
        _COMPILED["nc"] = _build_program()
    return _COMPILED["nc"]


def _prepare_in_maps(query, key_padding_mask, attn_mask, Wq, bq, Wk, bk, Wv,
                     bv):
    query = np.asarray(query, dtype=np.float32)
    attn_mask = np.asarray(attn_mask, dtype=np.float32)
    kpm = np.asarray(key_padding_mask)
    Wq, Wk, Wv = (np.asarray(w, dtype=np.float32) for w in (Wq, Wk, Wv))
    bq, bk, bv = (np.asarray(x, dtype=np.float32) for x in (bq, bk, bv))

    Wq = Wq * SCALE
    bq = bq * SCALE
    qt = np.ascontiguousarray(query.transpose(2, 1, 0))  # [E, B, T]
    # causal masks for the 4 diagonal-crossing tile offsets, from attn_mask:
    # cmh[d][p, f] = attn_mask[f, 128 d + p], f in [0, 512)
    cmh = np.stack(
        [
            np.ascontiguousarray(attn_mask[:512, 128 * d:128 * (d + 1)].T)
            for d in range(4)
        ]
    )
    cmh = np.maximum(cmh, NEG)  # -inf -> -1e30 (exp underflows to exactly 0)
    padf = np.where(kpm, NEG, 0.0).astype(np.float32)  # [B, T]
    padh = np.ascontiguousarray(
        padf.reshape(B, 8, 128).transpose(2, 0, 1).reshape(128, B * 8)
    )

    in_maps = []
    for i in range(NCORES):
        rows = slice(i * DS, (i + 1) * DS)
        in_maps.append(
            {
                "qt": qt,
                "wq": np.ascontiguousarray(Wq[rows].T),
                "wk": np.ascontiguousarray(Wk[rows].T),
                "wv": np.ascontiguousarray(Wv[rows].T),
                "bq": np.ascontiguousarray(bq[rows, None]),
                "bk": np.ascontiguousarray(bk[rows, None]),
                "bv": np.ascontiguousarray(bv[rows, None]),
                "cm": cmh,
                "pad": padh,
                "ones": np.ones((128, 16), dtype=np.float32),
            }
        )
    return in_maps


def kernel(query, key, key_padding_mask, attn_mask, Wq, bq, Wk, bk, Wv, bv,
           num_heads):
    from concourse.bass_utils import run_bass_kernel_spmd

    assert int(num_heads) == H
    nc = _get_program()
    in_maps = _prepare_in_maps(query, key_padding_mask, attn_mask, Wq, bq, Wk,
                               bk, Wv, bv)
    res = run_bass_kernel_spmd(nc, in_maps, core_ids=list(range(NCORES)))
    full = np.concatenate(
        [res.results[i]["out"] for i in range(NCORES)], axis=2
    ).transpose(1, 0, 2)
    return np.ascontiguousarray(full)


# revision 23
# speedup vs baseline: 1.0362x; 1.0362x over previous
"""Multihead self-attention (T=1024, B=4, E=1024, H=16) on 8 TRN2 NeuronCores.

Sharding: head-parallel. Core i owns heads {2i, 2i+1} == E-rows [128i, 128i+128)
of Wq/Wk/Wv, and all 4 batches. No cross-core communication.

Per-core dataflow (all "transposed" layouts, d on partitions):
  qT/kT/vT [128, B*T] = W_slice @ query.T   (PE, float32r, K=E in 8 chunks)
  per (b, head, t-chunk of 512):
    scoresT [s=128, t<=512] = kT_tile.T-free matmul; causal tiles above the
    diagonal are skipped entirely, diagonal tiles are column-sliced.
    probs = Exp(scoresT + causal_mask + padding_bias)  (ACT, padding as
    per-partition bias; both heads share one [128, 1024] ACT op)
    outT[65, 512] += va_tile[128, 65].T @ probs  where va has a ones column,
    so row 64 accumulates the softmax denominator.
  normalize: DMA-broadcast denominator row, DVE reciprocal + multiply.
Host gathers [128, B, T] per-core outputs -> [T, B, E].
"""

import numpy as np

T, B, E, H = 1024, 4, 1024, 16
D = 64  # head dim
NCORES = 8
HPC = H // NCORES  # heads per core = 2
DS = HPC * D  # per-core E-slice = 128
TB = T * B
NEG = -1.0e30
SCALE = D**-0.5

_COMPILED = {}


def _build_program():
    import concourse.bacc as bacc
    import concourse.mybir as mybir
    import concourse.tile as tile
    from concourse.masks import make_identity

    f32 = mybir.dt.float32
    f32r = mybir.dt.float32r
    AF = mybir.ActivationFunctionType
    ALU = mybir.AluOpType

    nc = bacc.Bacc("TRN2", target_bir_lowering=False, debug=False,
                   num_devices=NCORES)

    qt = nc.dram_tensor("qt", [E, B, T], f32r, kind="ExternalInput").ap()
    wq = nc.dram_tensor("wq", [E, DS], f32r, kind="ExternalInput").ap()
    wk = nc.dram_tensor("wk", [E, DS], f32r, kind="ExternalInput").ap()
    wv = nc.dram_tensor("wv", [E, DS], f32r, kind="ExternalInput").ap()
    bq = nc.dram_tensor("bq", [DS, 1], f32, kind="ExternalInput").ap()
    bk = nc.dram_tensor("bk", [DS, 1], f32, kind="ExternalInput").ap()
    bv = nc.dram_tensor("bv", [DS, 1], f32, kind="ExternalInput").ap()
    cm = nc.dram_tensor("cm", [4, 128, 512], f32, kind="ExternalInput").ap()
    pad = nc.dram_tensor("pad", [128, B * 8], f32, kind="ExternalInput").ap()
    onesd = nc.dram_tensor("ones", [128, 16], f32r, kind="ExternalInput").ap()
    out = nc.dram_tensor("out", [B, T, DS], f32, kind="ExternalOutput").ap()

    NJ = TB // 512  # 8 token chunks of 512; chunk j covers (b=j//2, half=j%2)

    with tile.TileContext(nc) as tc:
        with (
            tc.tile_pool(name="consts", bufs=1) as consts,
            tc.tile_pool(name="qkv", bufs=NJ) as qkv,
            tc.tile_pool(name="va", bufs=B) as vap,
            tc.tile_pool(name="probs", bufs=6) as probsp,
            tc.tile_pool(name="outsb", bufs=4) as outsb,
            tc.tile_pool(name="bcast", bufs=4) as bcastp,
        ):
            w_sb = {}
            b_sb = {}
            wdrs = {"q": wq, "k": wk, "v": wv}
            for nm in ("q", "k", "v"):
                w_sb[nm] = consts.tile([128, 8, DS], f32r, name=f"w{nm}s")
            # per-chunk weight loads so the first matmuls aren't gated on the
            # full 1.5 MB of weights
            for e in range(8):
                for nm in ("q", "k", "v"):
                    nc.scalar.dma_start(
                        w_sb[nm][:, e, :],
                        wdrs[nm].rearrange("(c p) m -> p c m", p=128)[:, e, :],
                    )
            for nm, bdr in (("q", bq), ("k", bk), ("v", bv)):
                bt = consts.tile([DS, 1], f32, name=f"b{nm}s")
                nc.scalar.dma_start(bt[:], bdr)
                b_sb[nm] = bt
            ident = consts.tile([128, 128], f32, name="ident")
            make_identity(nc, ident[:])
            cm_sb = consts.tile([128, 4, 512], f32, name="cms")
            nc.scalar.dma_start(cm_sb[:], cm.rearrange("d p f -> p d f"))
            pad_sb = consts.tile([128, B * 8], f32, name="pads")
            nc.scalar.dma_start(pad_sb[:], pad)

            # ---- Phase 1: QKV projection into transposed layout ----
            qkv_t = {"q": [], "k": [], "v": []}
            ph1 = tc.tile_pool(name="rhs", bufs=16)
            rhsp = ph1.__enter__()
            ph1b = tc.tile_pool(name="psA", bufs=2, space="PSUM")
            psA = ph1b.__enter__()
            for j in range(NJ):
                qu = []
                for e in range(8):
                    qtile = rhsp.tile([128, 512], f32r, tag="qu", name=f"qu{j}_{e}")
                    dma_eng = nc.sync if e % 2 == 0 else nc.scalar
                    dma_eng.dma_start(
                        qtile[:],
                        qt.rearrange("e b t -> e (b t)")[
                            e * 128:(e + 1) * 128, j * 512:(j + 1) * 512
                        ],
                    )
                    qu.append(qtile)
                for nm in ("q", "k", "v"):
                    ps = psA.tile([128, 512], f32, tag="proj", name=f"ps{nm}{j}")
                    for e in range(8):
                        nc.tensor.matmul(
                            ps[:],
                            w_sb[nm][:, e, :],
                            qu[e][:],
                            start=(e == 0),
                            stop=(e == 7),
                        )
                    dst = qkv.tile([128, 512], f32r if nm != "v" else f32, tag=f"{nm}t", name=f"{nm}t{j}")
                    nc.vector.tensor_scalar(
                        dst[:], ps[:], b_sb[nm][:], None, op0=ALU.add,
                    )
                    qkv_t[nm].append(dst)

            ph1b.__exit__(None, None, None)
            ph1.__exit__(None, None, None)

            # ---- Phase 1.5: transpose vT -> va ([s, d] with ones column) ----
            va_t = []
            for b in range(B):
                va = vap.tile([128, 8, 2, 65], f32r, tag="va", name=f"va{b}")
                nc.scalar.dma_start(
                    va[:, :, :, 64:65],
                    onesd.rearrange("p (a b c) -> p a b c", a=8, b=2, c=1),
                )
                va_t.append(va)
            ph15 = tc.tile_pool(name="psTr", bufs=2, space="PSUM")
            psTr = ph15.__enter__()
            for b in range(B):
                for p in range(8):
                    j = 2 * b + p // 4
                    off = (p % 4) * 128
                    tp = psTr.tile([128, 128], f32, tag="tr", name=f"tr{b}_{p}")
                    nc.tensor.transpose(
                        tp[:], qkv_t["v"][j][:, off:off + 128], ident[:]
                    )
                    nc.vector.tensor_copy(
                        va_t[b][:, p, :, 0:64],
                        tp[:].rearrange("p (two sub) -> p two sub", two=2),
                    )
            ph15.__exit__(None, None, None)

            # ---- Phase 2: attention (two batches interleaved so the PE
            # always has independent work while the exp chain drains) ----
            ph2 = tc.tile_pool(name="psS", bufs=2, space="PSUM")
            psS = ph2.__enter__()
            ph2b = tc.tile_pool(name="psO", bufs=4, space="PSUM")
            psO = ph2b.__enter__()
            for c in range(2):  # t-chunks of 512
                for bpair in ((0, 1), (2, 3)):
                    ntile = 4 * (c + 1)
                    po = {}
                    for bb in bpair:
                        for hl in range(HPC):
                            pot = psO.tile([65, 512], f32, tag="po",
                                           name=f"po{bb}_{c}_{hl}")
                            po[(bb, hl)] = pot
                    for p in range(ntile):
                        dlt = p - 4 * c
                        w0 = min(128 * dlt, 256) if dlt > 0 else 0
                        for bb in bpair:
                            j = 2 * bb + c
                            jk = 2 * bb + p // 4
                            offk = (p % 4) * 128
                            ss = psS.tile([128, 2, 512], f32, tag="sc",
                                          name=f"sc{bb}_{c}_{p}")
                            for hl in range(HPC):
                                nc.tensor.matmul(
                                    ss[:, hl, w0:512],
                                    qkv_t["k"][jk][hl * 64:(hl + 1) * 64,
                                                   offk:offk + 128],
                                    qkv_t["q"][j][hl * 64:(hl + 1) * 64,
                                                  w0:512],
                                    start=True,
                                    stop=True,
                                )
                            if dlt >= 0:
                                nc.vector.tensor_tensor(
                                    ss[:, :, w0:512],
                                    ss[:, :, w0:512],
                                    cm_sb[:, dlt, None, w0:512].to_broadcast(
                                        (128, 2, 512 - w0)
                                    ),
                                    ALU.add,
                                )
                            pr = probsp.tile([128, 2, 512], f32r, tag="pr",
                                             name=f"pr{bb}_{c}_{p}")
                            nc.scalar.activation(
                                pr[:, :, w0:512],
                                ss[:, :, w0:512],
                                AF.Exp,
                                bias=pad_sb[:, bb * 8 + p:bb * 8 + p + 1],
                                scale=1.0,
                            )
                            for hl in range(HPC):
                                nc.tensor.matmul(
                                    po[(bb, hl)][:, w0:512],
                                    va_t[bb][:, p, hl, :],
                                    pr[:, hl, w0:512],
                                    start=(p == 0),
                                    stop=(p == ntile - 1),
                                )
                    for bb in bpair:
                        for hl in range(HPC):
                            pos = bcastp.tile([65, 512], f32, tag="pos",
                                              name=f"pos{bb}_{c}_{hl}")
                            nc.scalar.activation(pos[:], po[(bb, hl)][:],
                                                 AF.Copy)
                            te = psO.tile([128, 4, 65], f32, tag="po",
                                          name=f"te{bb}_{c}_{hl}")
                            for g in range(4):
                                nc.tensor.transpose(
                                    te[:, g, :],
                                    pos[:, g * 128:(g + 1) * 128],
                                    ident[0:65, 0:65],
                                )
                            rcp = outsb.tile([128, 4, 1], f32, tag="rcp",
                                             name=f"rcp{bb}_{c}_{hl}")
                            nc.vector.reciprocal(rcp[:], te[:, :, 64:65])
                            of = outsb.tile([128, 4, 64], f32, tag="of",
                                            name=f"of{bb}_{c}_{hl}")
                            for g in range(4):
                                nc.vector.tensor_scalar(
                                    of[:, g, :], te[:, g, 0:64], rcp[:, g, :],
                                    None, op0=ALU.mult,
                                )
                            nc.gpsimd.dma_start(
                                out[bb, c * 512:(c + 1) * 512,
                                    hl * 64:(hl + 1) * 64].rearrange(
                                        "(g tp) m -> tp g m", tp=128),
                                of[:],
                            )
            ph2b.__exit__(None, None, None)
            ph2.__exit__(None, None, None)

    nc.compile()
    return nc


def _get_program():
    if "nc" not in _COMPILED:
        _COMPILED["nc"] = _build_program()
    return _COMPILED["nc"]


def _prepare_in_maps(query, key_padding_mask, attn_mask, Wq, bq, Wk, bk, Wv,
                     bv):
    query = np.asarray(query, dtype=np.float32)
    attn_mask = np.asarray(attn_mask, dtype=np.float32)
    kpm = np.asarray(key_padding_mask)
    Wq, Wk, Wv = (np.asarray(w, dtype=np.float32) for w in (Wq, Wk, Wv))
    bq, bk, bv = (np.asarray(x, dtype=np.float32) for x in (bq, bk, bv))

    Wq = Wq * SCALE
    bq = bq * SCALE
    qt = np.ascontiguousarray(query.transpose(2, 1, 0))  # [E, B, T]
    # causal masks for the 4 diagonal-crossing tile offsets, from attn_mask:
    # cmh[d][p, f] = attn_mask[f, 128 d + p], f in [0, 512)
    cmh = np.stack(
        [
            np.ascontiguousarray(attn_mask[:512, 128 * d:128 * (d + 1)].T)
            for d in range(4)
        ]
    )
    cmh = np.maximum(cmh, NEG)  # -inf -> -1e30 (exp underflows to exactly 0)
    padf = np.where(kpm, NEG, 0.0).astype(np.float32)  # [B, T]
    padh = np.ascontiguousarray(
        padf.reshape(B, 8, 128).transpose(2, 0, 1).reshape(128, B * 8)
    )

    in_maps = []
    for i in range(NCORES):
        rows = slice(i * DS, (i + 1) * DS)
        in_maps.append(
            {
                "qt": qt,
                "wq": np.ascontiguousarray(Wq[rows].T),
                "wk": np.ascontiguousarray(Wk[rows].T),
                "wv": np.ascontiguousarray(Wv[rows].T),
                "bq": np.ascontiguousarray(bq[rows, None]),
                "bk": np.ascontiguousarray(bk[rows, None]),
                "bv": np.ascontiguousarray(bv[rows, None]),
                "cm": cmh,
                "pad": padh,
                "ones": np.ones((128, 16), dtype=np.float32),
            }
        )
    return in_maps


def kernel(query, key, key_padding_mask, attn_mask, Wq, bq, Wk, bk, Wv, bv,
           num_heads):
    from concourse.bass_utils import run_bass_kernel_spmd

    assert int(num_heads) == H
    nc = _get_program()
    in_maps = _prepare_in_maps(query, key_padding_mask, attn_mask, Wq, bq, Wk,
                               bk, Wv, bv)
    res = run_bass_kernel_spmd(nc, in_maps, core_ids=list(range(NCORES)))
    full = np.concatenate(
        [res.results[i]["out"] for i in range(NCORES)], axis=2
    ).transpose(1, 0, 2)
    return np.ascontiguousarray(full)


# revision 24
# speedup vs baseline: 1.1472x; 1.1071x over previous
"""Multihead self-attention (T=1024, B=4, E=1024, H=16) on 8 TRN2 NeuronCores.

Sharding: head-parallel. Core i owns heads {2i, 2i+1} == E-rows [128i, 128i+128)
of Wq/Wk/Wv, and all 4 batches. No cross-core communication.

Per-core dataflow (all "transposed" layouts, d on partitions):
  qT/kT/vT [128, B*T] = W_slice @ query.T   (PE, float32r, K=E in 8 chunks)
  per (b, head, t-chunk of 512):
    scoresT [s=128, t<=512] = kT_tile.T-free matmul; causal tiles above the
    diagonal are skipped entirely, diagonal tiles are column-sliced.
    probs = Exp(scoresT + causal_mask + padding_bias)  (ACT, padding as
    per-partition bias; both heads share one [128, 1024] ACT op)
    outT[65, 512] += va_tile[128, 65].T @ probs  where va has a ones column,
    so row 64 accumulates the softmax denominator.
  normalize: DMA-broadcast denominator row, DVE reciprocal + multiply.
Host gathers [128, B, T] per-core outputs -> [T, B, E].
"""

import numpy as np

T, B, E, H = 1024, 4, 1024, 16
D = 64  # head dim
NCORES = 8
HPC = H // NCORES  # heads per core = 2
DS = HPC * D  # per-core E-slice = 128
TB = T * B
NEG = -1.0e30
SCALE = D**-0.5

_COMPILED = {}


def _build_program():
    import concourse.bacc as bacc
    import concourse.mybir as mybir
    import concourse.tile as tile
    from concourse.masks import make_identity

    f32 = mybir.dt.float32
    f32r = mybir.dt.float32r
    AF = mybir.ActivationFunctionType
    ALU = mybir.AluOpType

    nc = bacc.Bacc("TRN2", target_bir_lowering=False, debug=False,
                   num_devices=NCORES)

    qt = nc.dram_tensor("qt", [E, B, T], f32r, kind="ExternalInput").ap()
    wq = nc.dram_tensor("wq", [E, DS], f32r, kind="ExternalInput").ap()
    wk = nc.dram_tensor("wk", [E, DS], f32r, kind="ExternalInput").ap()
    wv = nc.dram_tensor("wv", [E, DS], f32r, kind="ExternalInput").ap()
    bq = nc.dram_tensor("bq", [DS, 1], f32, kind="ExternalInput").ap()
    bk = nc.dram_tensor("bk", [DS, 1], f32, kind="ExternalInput").ap()
    bv = nc.dram_tensor("bv", [DS, 1], f32, kind="ExternalInput").ap()
    cm = nc.dram_tensor("cm", [4, 128, 512], f32, kind="ExternalInput").ap()
    pad = nc.dram_tensor("pad", [128, B * 8], f32, kind="ExternalInput").ap()
    onesd = nc.dram_tensor("ones", [128, 16], f32r, kind="ExternalInput").ap()
    out = nc.dram_tensor("out", [B, T, DS], f32, kind="ExternalOutput").ap()

    NJ = TB // 512  # 8 token chunks of 512; chunk j covers (b=j//2, half=j%2)

    with tile.TileContext(nc) as tc:
        with (
            tc.tile_pool(name="consts", bufs=1) as consts,
            tc.tile_pool(name="qkv", bufs=NJ) as qkv,
            tc.tile_pool(name="va", bufs=B) as vap,
            tc.tile_pool(name="probs", bufs=6) as probsp,
            tc.tile_pool(name="outsb", bufs=4) as outsb,
            tc.tile_pool(name="bcast", bufs=4) as bcastp,
        ):
            w_sb = {}
            b_sb = {}
            wdrs = {"q": wq, "k": wk, "v": wv}
            for nm in ("q", "k", "v"):
                w_sb[nm] = consts.tile([128, 8, DS], f32r, name=f"w{nm}s")
            # per-chunk weight loads so the first matmuls aren't gated on the
            # full 1.5 MB of weights
            for e in range(8):
                for nm in ("q", "k", "v"):
                    nc.scalar.dma_start(
                        w_sb[nm][:, e, :],
                        wdrs[nm].rearrange("(c p) m -> p c m", p=128)[:, e, :],
                    )
            for nm, bdr in (("q", bq), ("k", bk), ("v", bv)):
                bt = consts.tile([DS, 1], f32, name=f"b{nm}s")
                nc.scalar.dma_start(bt[:], bdr)
                b_sb[nm] = bt
            ident = consts.tile([128, 128], f32, name="ident")
            make_identity(nc, ident[:])
            cm_sb = consts.tile([128, 4, 512], f32, name="cms")
            nc.scalar.dma_start(cm_sb[:], cm.rearrange("d p f -> p d f"))
            pad_sb = consts.tile([128, B * 8], f32, name="pads")
            nc.scalar.dma_start(pad_sb[:], pad)

            # ---- Phase 1: QKV projection into transposed layout ----
            qkv_t = {"q": [], "k": [], "v": []}
            ph1 = tc.tile_pool(name="rhs", bufs=3)
            rhsp = ph1.__enter__()
            ph1b = tc.tile_pool(name="psA", bufs=2, space="PSUM")
            psA = ph1b.__enter__()
            for j in range(NJ):
                qu = rhsp.tile([128, 8, 512], f32r, tag="qu", name=f"qu{j}")
                nc.sync.dma_start(
                    qu[:],
                    qt.rearrange("(c p) b t -> p c (b t)", p=128)[
                        :, :, j * 512:(j + 1) * 512
                    ],
                )
                for nm in ("q", "k", "v"):
                    ps = psA.tile([128, 512], f32, tag="proj", name=f"ps{nm}{j}")
                    for e in range(8):
                        nc.tensor.matmul(
                            ps[:],
                            w_sb[nm][:, e, :],
                            qu[:, e, :],
                            start=(e == 0),
                            stop=(e == 7),
                        )
                    dst = qkv.tile([128, 512], f32r if nm != "v" else f32, tag=f"{nm}t", name=f"{nm}t{j}")
                    nc.vector.tensor_scalar(
                        dst[:], ps[:], b_sb[nm][:], None, op0=ALU.add,
                    )
                    qkv_t[nm].append(dst)

            ph1b.__exit__(None, None, None)
            ph1.__exit__(None, None, None)

            # ---- Phase 1.5: transpose vT -> va ([s, d] with ones column) ----
            va_t = []
            for b in range(B):
                va = vap.tile([128, 8, 2, 65], f32r, tag="va", name=f"va{b}")
                nc.scalar.dma_start(
                    va[:, :, :, 64:65],
                    onesd.rearrange("p (a b c) -> p a b c", a=8, b=2, c=1),
                )
                va_t.append(va)
            ph15 = tc.tile_pool(name="psTr", bufs=2, space="PSUM")
            psTr = ph15.__enter__()
            for b in range(B):
                for p in range(8):
                    j = 2 * b + p // 4
                    off = (p % 4) * 128
                    tp = psTr.tile([128, 128], f32, tag="tr", name=f"tr{b}_{p}")
                    nc.tensor.transpose(
                        tp[:], qkv_t["v"][j][:, off:off + 128], ident[:]
                    )
                    nc.vector.tensor_copy(
                        va_t[b][:, p, :, 0:64],
                        tp[:].rearrange("p (two sub) -> p two sub", two=2),
                    )
            ph15.__exit__(None, None, None)

            # ---- Phase 2: attention (two batches interleaved so the PE
            # always has independent work while the exp chain drains) ----
            ph2 = tc.tile_pool(name="psS", bufs=2, space="PSUM")
            psS = ph2.__enter__()
            ph2b = tc.tile_pool(name="psO", bufs=4, space="PSUM")
            psO = ph2b.__enter__()
            for c in range(2):  # t-chunks of 512
                for bpair in ((0, 1), (2, 3)):
                    ntile = 4 * (c + 1)
                    po = {}
                    for bb in bpair:
                        for hl in range(HPC):
                            pot = psO.tile([65, 512], f32, tag="po",
                                           name=f"po{bb}_{c}_{hl}")
                            po[(bb, hl)] = pot
                    for p in range(ntile):
                        dlt = p - 4 * c
                        w0 = min(128 * dlt, 256) if dlt > 0 else 0
                        for bb in bpair:
                            j = 2 * bb + c
                            jk = 2 * bb + p // 4
                            offk = (p % 4) * 128
                            ss = psS.tile([128, 2, 512], f32, tag="sc",
                                          name=f"sc{bb}_{c}_{p}")
                            for hl in range(HPC):
                                nc.tensor.matmul(
                                    ss[:, hl, w0:512],
                                    qkv_t["k"][jk][hl * 64:(hl + 1) * 64,
                                                   offk:offk + 128],
                                    qkv_t["q"][j][hl * 64:(hl + 1) * 64,
                                                  w0:512],
                                    start=True,
                                    stop=True,
                                )
                            if dlt >= 0:
                                nc.vector.tensor_tensor(
                                    ss[:, :, w0:512],
                                    ss[:, :, w0:512],
                                    cm_sb[:, dlt, None, w0:512].to_broadcast(
                                        (128, 2, 512 - w0)
                                    ),
                                    ALU.add,
                                )
                            pr = probsp.tile([128, 2, 512], f32r, tag="pr",
                                             name=f"pr{bb}_{c}_{p}")
                            nc.scalar.activation(
                                pr[:, :, w0:512],
                                ss[:, :, w0:512],
                                AF.Exp,
                                bias=pad_sb[:, bb * 8 + p:bb * 8 + p + 1],
                                scale=1.0,
                            )
                            for hl in range(HPC):
                                nc.tensor.matmul(
                                    po[(bb, hl)][:, w0:512],
                                    va_t[bb][:, p, hl, :],
                                    pr[:, hl, w0:512],
                                    start=(p == 0),
                                    stop=(p == ntile - 1),
                                )
                    for bb in bpair:
                        for hl in range(HPC):
                            pos = bcastp.tile([65, 512], f32, tag="pos",
                                              name=f"pos{bb}_{c}_{hl}")
                            nc.scalar.activation(pos[:], po[(bb, hl)][:],
                                                 AF.Copy)
                            te = psO.tile([128, 4, 65], f32, tag="po",
                                          name=f"te{bb}_{c}_{hl}")
                            for g in range(4):
                                nc.tensor.transpose(
                                    te[:, g, :],
                                    pos[:, g * 128:(g + 1) * 128],
                                    ident[0:65, 0:65],
                                )
                            rcp = outsb.tile([128, 4, 1], f32, tag="rcp",
                                             name=f"rcp{bb}_{c}_{hl}")
                            nc.vector.reciprocal(rcp[:], te[:, :, 64:65])
                            of = outsb.tile([128, 4, 64], f32, tag="of",
                                            name=f"of{bb}_{c}_{hl}")
                            for g in range(4):
                                nc.vector.tensor_scalar(
                                    of[:, g, :], te[:, g, 0:64], rcp[:, g, :],
                                    None, op0=ALU.mult,
                                )
                            nc.gpsimd.dma_start(
                                out[bb, c * 512:(c + 1) * 512,
                                    hl * 64:(hl + 1) * 64].rearrange(
                                        "(g tp) m -> tp g m", tp=128),
                                of[:],
                            )
            ph2b.__exit__(None, None, None)
            ph2.__exit__(None, None, None)

    nc.compile()
    return nc


def _get_program():
    if "nc" not in _COMPILED:
        _COMPILED["nc"] = _build_program()
    return _COMPILED["nc"]


def _prepare_in_maps(query, key_padding_mask, attn_mask, Wq, bq, Wk, bk, Wv,
                     bv):
    query = np.asarray(query, dtype=np.float32)
    attn_mask = np.asarray(attn_mask, dtype=np.float32)
    kpm = np.asarray(key_padding_mask)
    Wq, Wk, Wv = (np.asarray(w, dtype=np.float32) for w in (Wq, Wk, Wv))
    bq, bk, bv = (np.asarray(x, dtype=np.float32) for x in (bq, bk, bv))

    Wq = Wq * SCALE
    bq = bq * SCALE
    qt = np.ascontiguousarray(query.transpose(2, 1, 0))  # [E, B, T]
    # causal masks for the 4 diagonal-crossing tile offsets, from attn_mask:
    # cmh[d][p, f] = attn_mask[f, 128 d + p], f in [0, 512)
    cmh = np.stack(
        [
            np.ascontiguousarray(attn_mask[:512, 128 * d:128 * (d + 1)].T)
            for d in range(4)
        ]
    )
    cmh = np.maximum(cmh, NEG)  # -inf -> -1e30 (exp underflows to exactly 0)
    padf = np.where(kpm, NEG, 0.0).astype(np.float32)  # [B, T]
    padh = np.ascontiguousarray(
        padf.reshape(B, 8, 128).transpose(2, 0, 1).reshape(128, B * 8)
    )

    in_maps = []
    for i in range(NCORES):
        rows = slice(i * DS, (i + 1) * DS)
        in_maps.append(
            {
                "qt": qt,
                "wq": np.ascontiguousarray(Wq[rows].T),
                "wk": np.ascontiguousarray(Wk[rows].T),
                "wv": np.ascontiguousarray(Wv[rows].T),
                "bq": np.ascontiguousarray(bq[rows, None]),
                "bk": np.ascontiguousarray(bk[rows, None]),
                "bv": np.ascontiguousarray(bv[rows, None]),
                "cm": cmh,
                "pad": padh,
                "ones": np.ones((128, 16), dtype=np.float32),
            }
        )
    return in_maps


def kernel(query, key, key_padding_mask, attn_mask, Wq, bq, Wk, bk, Wv, bv,
           num_heads):
    from concourse.bass_utils import run_bass_kernel_spmd

    assert int(num_heads) == H
    nc = _get_program()
    in_maps = _prepare_in_maps(query, key_padding_mask, attn_mask, Wq, bq, Wk,
                               bk, Wv, bv)
    res = run_bass_kernel_spmd(nc, in_maps, core_ids=list(range(NCORES)))
    full = np.concatenate(
        [res.results[i]["out"] for i in range(NCORES)], axis=2
    ).transpose(1, 0, 2)
    return np.ascontiguousarray(full)


# revision 26
# speedup vs baseline: 1.2308x; 1.0729x over previous
"""Multihead self-attention (T=1024, B=4, E=1024, H=16) on 8 TRN2 NeuronCores.

Sharding: head-parallel. Core i owns heads {2i, 2i+1} == E-rows [128i, 128i+128)
of Wq/Wk/Wv, and all 4 batches. No cross-core communication.

Per-core dataflow (all "transposed" layouts, d on partitions):
  qT/kT/vT [128, B*T] = W_slice @ query.T   (PE, float32r, K=E in 8 chunks)
  per (b, head, t-chunk of 512):
    scoresT [s=128, t<=512] = kT_tile.T-free matmul; causal tiles above the
    diagonal are skipped entirely, diagonal tiles are column-sliced.
    probs = Exp(scoresT + causal_mask + padding_bias)  (ACT, padding as
    per-partition bias; both heads share one [128, 1024] ACT op)
    outT[65, 512] += va_tile[128, 65].T @ probs  where va has a ones column,
    so row 64 accumulates the softmax denominator.
  normalize: DMA-broadcast denominator row, DVE reciprocal + multiply.
Host gathers [128, B, T] per-core outputs -> [T, B, E].
"""

import numpy as np

T, B, E, H = 1024, 4, 1024, 16
D = 64  # head dim
NCORES = 8
HPC = H // NCORES  # heads per core = 2
DS = HPC * D  # per-core E-slice = 128
TB = T * B
NEG = -1.0e30
SCALE = D**-0.5

_COMPILED = {}


def _build_program():
    import concourse.bacc as bacc
    import concourse.mybir as mybir
    import concourse.tile as tile
    from concourse.masks import make_identity

    f32 = mybir.dt.float32
    f32r = mybir.dt.float32r
    AF = mybir.ActivationFunctionType
    ALU = mybir.AluOpType

    nc = bacc.Bacc("TRN2", target_bir_lowering=False, debug=False,
                   num_devices=NCORES)

    qt = nc.dram_tensor("qt", [E, B, T], f32r, kind="ExternalInput").ap()
    wq = nc.dram_tensor("wq", [E, DS], f32r, kind="ExternalInput").ap()
    wk = nc.dram_tensor("wk", [E, DS], f32r, kind="ExternalInput").ap()
    wv = nc.dram_tensor("wv", [E, DS], f32r, kind="ExternalInput").ap()
    bq = nc.dram_tensor("bq", [DS, 1], f32, kind="ExternalInput").ap()
    bk = nc.dram_tensor("bk", [DS, 1], f32, kind="ExternalInput").ap()
    bv = nc.dram_tensor("bv", [DS, 1], f32, kind="ExternalInput").ap()
    cm = nc.dram_tensor("cm", [4, 128, 512], f32, kind="ExternalInput").ap()
    pad = nc.dram_tensor("pad", [128, B * 8], f32, kind="ExternalInput").ap()
    onesd = nc.dram_tensor("ones", [128, 16], f32r, kind="ExternalInput").ap()
    out = nc.dram_tensor("out", [B, T, DS], f32, kind="ExternalOutput").ap()

    NJ = TB // 512  # 8 token chunks of 512; chunk j covers (b=j//2, half=j%2)

    with tile.TileContext(nc) as tc:
        with (
            tc.tile_pool(name="consts", bufs=1) as consts,
            tc.tile_pool(name="qkv", bufs=NJ) as qkv,
            tc.tile_pool(name="va", bufs=B) as vap,
        ):
            w_sb = {}
            b_sb = {}
            wdrs = {"q": wq, "k": wk, "v": wv}
            for nm in ("q", "k", "v"):
                w_sb[nm] = consts.tile([128, 8, DS], f32r, name=f"w{nm}s")
            # per-chunk weight loads so the first matmuls aren't gated on the
            # full 1.5 MB of weights
            for e in range(8):
                for nm in ("q", "k", "v"):
                    nc.scalar.dma_start(
                        w_sb[nm][:, e, :],
                        wdrs[nm].rearrange("(c p) m -> p c m", p=128)[:, e, :],
                    )
            for nm, bdr in (("q", bq), ("k", bk), ("v", bv)):
                bt = consts.tile([DS, 1], f32, name=f"b{nm}s")
                nc.scalar.dma_start(bt[:], bdr)
                b_sb[nm] = bt
            ident = consts.tile([128, 128], f32, name="ident")
            make_identity(nc, ident[:])
            cm_sb = consts.tile([128, 4, 512], f32, name="cms")
            nc.scalar.dma_start(cm_sb[:], cm.rearrange("d p f -> p d f"))
            pad_sb = consts.tile([128, B * 8], f32, name="pads")
            nc.scalar.dma_start(pad_sb[:], pad)

            # ---- Phase 1: QKV projection into transposed layout ----
            qkv_t = {"q": [], "k": [], "v": []}
            ph1 = tc.tile_pool(name="rhs", bufs=3)
            rhsp = ph1.__enter__()
            ph1b = tc.tile_pool(name="psA", bufs=2, space="PSUM")
            psA = ph1b.__enter__()
            for j in range(NJ):
                qu = rhsp.tile([128, 8, 512], f32r, tag="qu", name=f"qu{j}")
                nc.sync.dma_start(
                    qu[:],
                    qt.rearrange("(c p) b t -> p c (b t)", p=128)[
                        :, :, j * 512:(j + 1) * 512
                    ],
                )
                for nm in ("q", "k", "v"):
                    ps = psA.tile([128, 512], f32, tag="proj", name=f"ps{nm}{j}")
                    for e in range(8):
                        nc.tensor.matmul(
                            ps[:],
                            w_sb[nm][:, e, :],
                            qu[:, e, :],
                            start=(e == 0),
                            stop=(e == 7),
                        )
                    dst = qkv.tile([128, 512], f32r if nm != "v" else f32, tag=f"{nm}t", name=f"{nm}t{j}")
                    nc.vector.tensor_scalar(
                        dst[:], ps[:], b_sb[nm][:], None, op0=ALU.add,
                    )
                    qkv_t[nm].append(dst)

            ph1b.__exit__(None, None, None)
            ph1.__exit__(None, None, None)

            # ---- Phase 1.5: transpose vT -> va ([s, d] with ones column) ----
            va_t = []
            for b in range(B):
                va = vap.tile([128, 8, 2, 65], f32r, tag="va", name=f"va{b}")
                nc.scalar.dma_start(
                    va[:, :, :, 64:65],
                    onesd.rearrange("p (a b c) -> p a b c", a=8, b=2, c=1),
                )
                va_t.append(va)
            ph15 = tc.tile_pool(name="psTr", bufs=2, space="PSUM")
            psTr = ph15.__enter__()
            for b in range(B):
                for p in range(8):
                    j = 2 * b + p // 4
                    off = (p % 4) * 128
                    tp = psTr.tile([128, 128], f32, tag="tr", name=f"tr{b}_{p}")
                    nc.tensor.transpose(
                        tp[:], qkv_t["v"][j][:, off:off + 128], ident[:]
                    )
                    nc.vector.tensor_copy(
                        va_t[b][:, p, :, 0:64],
                        tp[:].rearrange("p (two sub) -> p two sub", two=2),
                    )
            ph15.__exit__(None, None, None)

            # ---- Phase 2: attention, software-pipelined at chunk level:
            # scores+exp of chunk k run on PE/ACT while AV matmuls of chunk
            # k-1 (whose probs are already in SBUF) fill the PE's stall slots.
            ph2 = tc.tile_pool(name="psS", bufs=2, space="PSUM")
            psS = ph2.__enter__()
            ph2b = tc.tile_pool(name="psO", bufs=4, space="PSUM")
            psO = ph2b.__enter__()
            ph2p = tc.tile_pool(name="probs", bufs=14)
            probsp = ph2p.__enter__()
            ph2o = tc.tile_pool(name="outsb", bufs=4)
            outsb = ph2o.__enter__()
            ph2q = tc.tile_pool(name="bcast", bufs=4)
            bcastp = ph2q.__enter__()

            po = {}

            def w0_of(c, p):
                dlt = p - 4 * c
                return (min(128 * dlt, 256) if dlt > 0 else 0), dlt

            def emit_scores_exp(b, c, p):
                w0, dlt = w0_of(c, p)
                j = 2 * b + c
                jk = 2 * b + p // 4
                offk = (p % 4) * 128
                ss = psS.tile([128, 2, 512], f32, tag="sc",
                              name=f"sc{b}_{c}_{p}")
                for hl in range(HPC):
                    nc.tensor.matmul(
                        ss[:, hl, w0:512],
                        qkv_t["k"][jk][hl * 64:(hl + 1) * 64,
                                       offk:offk + 128],
                        qkv_t["q"][j][hl * 64:(hl + 1) * 64, w0:512],
                        start=True,
                        stop=True,
                    )
                if dlt >= 0:
                    nc.vector.tensor_tensor(
                        ss[:, :, w0:512],
                        ss[:, :, w0:512],
                        cm_sb[:, dlt, None, w0:512].to_broadcast(
                            (128, 2, 512 - w0)),
                        ALU.add,
                    )
                pr = probsp.tile([128, 2, 512], f32r, tag="pr",
                                 name=f"pr{b}_{c}_{p}")
                nc.scalar.activation(
                    pr[:, :, w0:512],
                    ss[:, :, w0:512],
                    AF.Exp,
                    bias=pad_sb[:, b * 8 + p:b * 8 + p + 1],
                    scale=1.0,
                )
                return pr

            def emit_av(b, c, p, pr):
                w0, _ = w0_of(c, p)
                ntile = 4 * (c + 1)
                for hl in range(HPC):
                    nc.tensor.matmul(
                        po[(b, c)][hl][:, w0:512],
                        va_t[b][:, p, hl, :],
                        pr[:, hl, w0:512],
                        start=(p == 0),
                        stop=(p == ntile - 1),
                    )

            def emit_epilogue(b, c):
                for hl in range(HPC):
                    pos = bcastp.tile([65, 512], f32, tag="pos",
                                      name=f"pos{b}_{c}_{hl}")
                    nc.scalar.activation(pos[:], po[(b, c)][hl][:], AF.Copy)
                    te = psO.tile([128, 4, 65], f32, tag="po",
                                  name=f"te{b}_{c}_{hl}")
                    for g in range(4):
                        nc.tensor.transpose(
                            te[:, g, :],
                            pos[:, g * 128:(g + 1) * 128],
                            ident[0:65, 0:65],
                        )
                    rcp = outsb.tile([128, 4, 1], f32, tag="rcp",
                                     name=f"rcp{b}_{c}_{hl}")
                    nc.vector.reciprocal(rcp[:], te[:, :, 64:65])
                    of = outsb.tile([128, 4, 64], f32, tag="of",
                                    name=f"of{b}_{c}_{hl}")
                    for g in range(4):
                        nc.vector.tensor_scalar(
                            of[:, g, :], te[:, g, 0:64], rcp[:, g, :],
                            None, op0=ALU.mult,
                        )
                    nc.gpsimd.dma_start(
                        out[b, c * 512:(c + 1) * 512,
                            hl * 64:(hl + 1) * 64].rearrange(
                                "(g tp) m -> tp g m", tp=128),
                        of[:],
                    )

            chunks = [(b, c) for b in range(B) for c in range(2)]
            prev = None
            prev_pending = []
            for (b, c) in chunks:
                ntile = 4 * (c + 1)
                po[(b, c)] = [
                    psO.tile([65, 512], f32, tag="po", name=f"po{b}_{c}_{hl}")
                    for hl in range(HPC)
                ]
                prs = []
                for p in range(ntile):
                    pr = emit_scores_exp(b, c, p)
                    if prev_pending:
                        emit_av(*prev_pending.pop(0))
                    prs.append((b, c, p, pr))
                while prev_pending:
                    emit_av(*prev_pending.pop(0))
                if prev is not None:
                    emit_epilogue(*prev)
                prev = (b, c)
                prev_pending = prs
            while prev_pending:
                emit_av(*prev_pending.pop(0))
            emit_epilogue(*prev)

            ph2q.__exit__(None, None, None)
            ph2o.__exit__(None, None, None)
            ph2p.__exit__(None, None, None)
            ph2b.__exit__(None, None, None)
            ph2.__exit__(None, None, None)

    nc.compile()
    return nc


def _get_program():
    if "nc" not in _COMPILED:
        _COMPILED["nc"] = _build_program()
    return _COMPILED["nc"]


def _prepare_in_maps(query, key_padding_mask, attn_mask, Wq, bq, Wk, bk, Wv,
                     bv):
    query = np.asarray(query, dtype=np.float32)
    attn_mask = np.asarray(attn_mask, dtype=np.float32)
    kpm = np.asarray(key_padding_mask)
    Wq, Wk, Wv = (np.asarray(w, dtype=np.float32) for w in (Wq, Wk, Wv))
    bq, bk, bv = (np.asarray(x, dtype=np.float32) for x in (bq, bk, bv))

    Wq = Wq * SCALE
    bq = bq * SCALE
    qt = np.ascontiguousarray(query.transpose(2, 1, 0))  # [E, B, T]
    # causal masks for the 4 diagonal-crossing tile offsets, from attn_mask:
    # cmh[d][p, f] = attn_mask[f, 128 d + p], f in [0, 512)
    cmh = np.stack(
        [
            np.ascontiguousarray(attn_mask[:512, 128 * d:128 * (d + 1)].T)
            for d in range(4)
        ]
    )
    cmh = np.maximum(cmh, NEG)  # -inf -> -1e30 (exp underflows to exactly 0)
    padf = np.where(kpm, NEG, 0.0).astype(np.float32)  # [B, T]
    padh = np.ascontiguousarray(
        padf.reshape(B, 8, 128).transpose(2, 0, 1).reshape(128, B * 8)
    )

    in_maps = []
    for i in range(NCORES):
        rows = slice(i * DS, (i + 1) * DS)
        in_maps.append(
            {
                "qt": qt,
                "wq": np.ascontiguousarray(Wq[rows].T),
                "wk": np.ascontiguousarray(Wk[rows].T),
                "wv": np.ascontiguousarray(Wv[rows].T),
                "bq": np.ascontiguousarray(bq[rows, None]),
                "bk": np.ascontiguousarray(bk[rows, None]),
                "bv": np.ascontiguousarray(bv[rows, None]),
                "cm": cmh,
                "pad": padh,
                "ones": np.ones((128, 16), dtype=np.float32),
            }
        )
    return in_maps


def kernel(query, key, key_padding_mask, attn_mask, Wq, bq, Wk, bk, Wv, bv,
           num_heads):
    from concourse.bass_utils import run_bass_kernel_spmd

    assert int(num_heads) == H
    nc = _get_program()
    in_maps = _prepare_in_maps(query, key_padding_mask, attn_mask, Wq, bq, Wk,
                               bk, Wv, bv)
    res = run_bass_kernel_spmd(nc, in_maps, core_ids=list(range(NCORES)))
    full = np.concatenate(
        [res.results[i]["out"] for i in range(NCORES)], axis=2
    ).transpose(1, 0, 2)
    return np.ascontiguousarray(full)


# revision 27
# speedup vs baseline: 1.2713x; 1.0329x over previous
"""Multihead self-attention (T=1024, B=4, E=1024, H=16) on 8 TRN2 NeuronCores.

Sharding: head-parallel. Core i owns heads {2i, 2i+1} == E-rows [128i, 128i+128)
of Wq/Wk/Wv, and all 4 batches. No cross-core communication.

Per-core dataflow (all "transposed" layouts, d on partitions):
  qT/kT/vT [128, B*T] = W_slice @ query.T   (PE, float32r, K=E in 8 chunks)
  per (b, head, t-chunk of 512):
    scoresT [s=128, t<=512] = kT_tile.T-free matmul; causal tiles above the
    diagonal are skipped entirely, diagonal tiles are column-sliced.
    probs = Exp(scoresT + causal_mask + padding_bias)  (ACT, padding as
    per-partition bias; both heads share one [128, 1024] ACT op)
    outT[65, 512] += va_tile[128, 65].T @ probs  where va has a ones column,
    so row 64 accumulates the softmax denominator.
  normalize: DMA-broadcast denominator row, DVE reciprocal + multiply.
Host gathers [128, B, T] per-core outputs -> [T, B, E].
"""

import numpy as np

T, B, E, H = 1024, 4, 1024, 16
D = 64  # head dim
NCORES = 8
HPC = H // NCORES  # heads per core = 2
DS = HPC * D  # per-core E-slice = 128
TB = T * B
NEG = -1.0e30
SCALE = D**-0.5

_COMPILED = {}


def _build_program():
    import concourse.bacc as bacc
    import concourse.mybir as mybir
    import concourse.tile as tile
    from concourse.masks import make_identity

    f32 = mybir.dt.float32
    f32r = mybir.dt.float32r
    AF = mybir.ActivationFunctionType
    ALU = mybir.AluOpType

    nc = bacc.Bacc("TRN2", target_bir_lowering=False, debug=False,
                   num_devices=NCORES)

    qt = nc.dram_tensor("qt", [E, B, T], f32r, kind="ExternalInput").ap()
    wq = nc.dram_tensor("wq", [E, DS], f32r, kind="ExternalInput").ap()
    wk = nc.dram_tensor("wk", [E, DS], f32r, kind="ExternalInput").ap()
    wv = nc.dram_tensor("wv", [E, DS], f32r, kind="ExternalInput").ap()
    bq = nc.dram_tensor("bq", [DS, 1], f32, kind="ExternalInput").ap()
    bk = nc.dram_tensor("bk", [DS, 1], f32, kind="ExternalInput").ap()
    bv = nc.dram_tensor("bv", [DS, 1], f32, kind="ExternalInput").ap()
    cm = nc.dram_tensor("cm", [4, 128, 512], f32, kind="ExternalInput").ap()
    pad = nc.dram_tensor("pad", [128, B * 8], f32, kind="ExternalInput").ap()
    onesd = nc.dram_tensor("ones", [128, 16], f32r, kind="ExternalInput").ap()
    out = nc.dram_tensor("out", [B, T, DS], f32, kind="ExternalOutput").ap()

    NJ = TB // 512  # 8 token chunks of 512; chunk j covers (b=j//2, half=j%2)

    with tile.TileContext(nc) as tc:
        with (
            tc.tile_pool(name="consts", bufs=1) as consts,
            tc.tile_pool(name="qkv", bufs=NJ) as qkv,
            tc.tile_pool(name="va", bufs=B) as vap,
        ):
            w_sb = {}
            b_sb = {}
            wdrs = {"q": wq, "k": wk, "v": wv}
            for nm in ("q", "k", "v"):
                w_sb[nm] = consts.tile([128, 8, DS], f32r, name=f"w{nm}s")
            for nm in ("q", "k", "v"):
                nc.scalar.dma_start(
                    w_sb[nm][:],
                    wdrs[nm].rearrange("(c p) m -> p c m", p=128),
                )
            for nm, bdr in (("q", bq), ("k", bk), ("v", bv)):
                bt = consts.tile([DS, 1], f32, name=f"b{nm}s")
                nc.scalar.dma_start(bt[:], bdr)
                b_sb[nm] = bt
            ident = consts.tile([128, 128], f32, name="ident")
            make_identity(nc, ident[:])
            cm_sb = consts.tile([128, 4, 512], f32, name="cms")
            nc.scalar.dma_start(cm_sb[:], cm.rearrange("d p f -> p d f"))
            pad_sb = consts.tile([128, B * 8], f32, name="pads")
            nc.scalar.dma_start(pad_sb[:], pad)

            # ---- Phase 1: QKV projection into transposed layout ----
            qkv_t = {"q": [], "k": [], "v": []}
            ph1 = tc.tile_pool(name="rhs", bufs=3)
            rhsp = ph1.__enter__()
            ph1b = tc.tile_pool(name="psA", bufs=2, space="PSUM")
            psA = ph1b.__enter__()
            for j in range(NJ):
                qu = rhsp.tile([128, 8, 512], f32r, tag="qu", name=f"qu{j}")
                nc.sync.dma_start(
                    qu[:],
                    qt.rearrange("(c p) b t -> p c (b t)", p=128)[
                        :, :, j * 512:(j + 1) * 512
                    ],
                )
                for nm in ("q", "k", "v"):
                    ps = psA.tile([128, 512], f32, tag="proj", name=f"ps{nm}{j}")
                    for e in range(8):
                        nc.tensor.matmul(
                            ps[:],
                            w_sb[nm][:, e, :],
                            qu[:, e, :],
                            start=(e == 0),
                            stop=(e == 7),
                        )
                    dst = qkv.tile([128, 512], f32r if nm != "v" else f32, tag=f"{nm}t", name=f"{nm}t{j}")
                    nc.vector.tensor_scalar(
                        dst[:], ps[:], b_sb[nm][:], None, op0=ALU.add,
                    )
                    qkv_t[nm].append(dst)

            ph1b.__exit__(None, None, None)
            ph1.__exit__(None, None, None)

            # ---- Phase 1.5: transpose vT -> va ([s, d] with ones column) ----
            va_t = []
            for b in range(B):
                va = vap.tile([128, 8, 2, 65], f32r, tag="va", name=f"va{b}")
                nc.scalar.dma_start(
                    va[:, :, :, 64:65],
                    onesd.rearrange("p (a b c) -> p a b c", a=8, b=2, c=1),
                )
                va_t.append(va)
            ph15 = tc.tile_pool(name="psTr", bufs=2, space="PSUM")
            psTr = ph15.__enter__()
            for b in range(B):
                for p in range(8):
                    j = 2 * b + p // 4
                    off = (p % 4) * 128
                    tp = psTr.tile([128, 128], f32, tag="tr", name=f"tr{b}_{p}")
                    nc.tensor.transpose(
                        tp[:], qkv_t["v"][j][:, off:off + 128], ident[:]
                    )
                    nc.vector.tensor_copy(
                        va_t[b][:, p, :, 0:64],
                        tp[:].rearrange("p (two sub) -> p two sub", two=2),
                    )
            ph15.__exit__(None, None, None)

            # ---- Phase 2: attention, software-pipelined at chunk level:
            # scores+exp of chunk k run on PE/ACT while AV matmuls of chunk
            # k-1 (whose probs are already in SBUF) fill the PE's stall slots.
            ph2 = tc.tile_pool(name="psS", bufs=2, space="PSUM")
            psS = ph2.__enter__()
            ph2b = tc.tile_pool(name="psO", bufs=4, space="PSUM")
            psO = ph2b.__enter__()
            ph2p = tc.tile_pool(name="probs", bufs=14)
            probsp = ph2p.__enter__()
            ph2o = tc.tile_pool(name="outsb", bufs=4)
            outsb = ph2o.__enter__()
            ph2q = tc.tile_pool(name="bcast", bufs=4)
            bcastp = ph2q.__enter__()

            po = {}

            def w0_of(c, p):
                dlt = p - 4 * c
                return (min(128 * dlt, 256) if dlt > 0 else 0), dlt

            def emit_scores_exp(b, c, p):
                w0, dlt = w0_of(c, p)
                j = 2 * b + c
                jk = 2 * b + p // 4
                offk = (p % 4) * 128
                ss = psS.tile([128, 2, 512], f32, tag="sc",
                              name=f"sc{b}_{c}_{p}")
                for hl in range(HPC):
                    nc.tensor.matmul(
                        ss[:, hl, w0:512],
                        qkv_t["k"][jk][hl * 64:(hl + 1) * 64,
                                       offk:offk + 128],
                        qkv_t["q"][j][hl * 64:(hl + 1) * 64, w0:512],
                        start=True,
                        stop=True,
                    )
                if dlt >= 0:
                    nc.vector.tensor_tensor(
                        ss[:, :, w0:512],
                        ss[:, :, w0:512],
                        cm_sb[:, dlt, None, w0:512].to_broadcast(
                            (128, 2, 512 - w0)),
                        ALU.add,
                    )
                pr = probsp.tile([128, 2, 512], f32r, tag="pr",
                                 name=f"pr{b}_{c}_{p}")
                nc.scalar.activation(
                    pr[:, :, w0:512],
                    ss[:, :, w0:512],
                    AF.Exp,
                    bias=pad_sb[:, b * 8 + p:b * 8 + p + 1],
                    scale=1.0,
                )
                return pr

            def emit_av(b, c, p, pr):
                w0, _ = w0_of(c, p)
                ntile = 4 * (c + 1)
                for hl in range(HPC):
                    nc.tensor.matmul(
                        po[(b, c)][hl][:, w0:512],
                        va_t[b][:, p, hl, :],
                        pr[:, hl, w0:512],
                        start=(p == 0),
                        stop=(p == ntile - 1),
                    )

            def emit_epilogue(b, c):
                for hl in range(HPC):
                    pos = bcastp.tile([65, 512], f32, tag="pos",
                                      name=f"pos{b}_{c}_{hl}")
                    nc.vector.tensor_copy(pos[:], po[(b, c)][hl][:])
                    te = psO.tile([128, 4, 65], f32, tag="po",
                                  name=f"te{b}_{c}_{hl}")
                    for g in range(4):
                        nc.tensor.transpose(
                            te[:, g, :],
                            pos[:, g * 128:(g + 1) * 128],
                            ident[0:65, 0:65],
                        )
                    rcp = outsb.tile([128, 4, 1], f32, tag="rcp",
                                     name=f"rcp{b}_{c}_{hl}")
                    nc.vector.reciprocal(rcp[:], te[:, :, 64:65])
                    of = outsb.tile([128, 4, 64], f32, tag="of",
                                    name=f"of{b}_{c}_{hl}")
                    for g in range(4):
                        nc.vector.tensor_scalar(
                            of[:, g, :], te[:, g, 0:64], rcp[:, g, :],
                            None, op0=ALU.mult,
                        )
                    nc.gpsimd.dma_start(
                        out[b, c * 512:(c + 1) * 512,
                            hl * 64:(hl + 1) * 64].rearrange(
                                "(g tp) m -> tp g m", tp=128),
                        of[:],
                    )

            chunks = [(b, c) for b in range(B) for c in range(2)]
            prev = None
            prev_pending = []
            for (b, c) in chunks:
                ntile = 4 * (c + 1)
                po[(b, c)] = [
                    psO.tile([65, 512], f32, tag="po", name=f"po{b}_{c}_{hl}")
                    for hl in range(HPC)
                ]
                prs = []
                for p in range(ntile):
                    pr = emit_scores_exp(b, c, p)
                    if prev_pending:
                        emit_av(*prev_pending.pop(0))
                    if p == 2 and prev is not None:
                        while prev_pending:
                            emit_av(*prev_pending.pop(0))
                        emit_epilogue(*prev)
                        prev = None
                    prs.append((b, c, p, pr))
                while prev_pending:
                    emit_av(*prev_pending.pop(0))
                if prev is not None:
                    emit_epilogue(*prev)
                prev = (b, c)
                prev_pending = prs
            while prev_pending:
                emit_av(*prev_pending.pop(0))
            emit_epilogue(*prev)

            ph2q.__exit__(None, None, None)
            ph2o.__exit__(None, None, None)
            ph2p.__exit__(None, None, None)
            ph2b.__exit__(None, None, None)
            ph2.__exit__(None, None, None)

    nc.compile()
    return nc


def _get_program():
    if "nc" not in _COMPILED:
        _COMPILED["nc"] = _build_program()
    return _COMPILED["nc"]


def _prepare_in_maps(query, key_padding_mask, attn_mask, Wq, bq, Wk, bk, Wv,
                     bv):
    query = np.asarray(query, dtype=np.float32)
    attn_mask = np.asarray(attn_mask, dtype=np.float32)
    kpm = np.asarray(key_padding_mask)
    Wq, Wk, Wv = (np.asarray(w, dtype=np.float32) for w in (Wq, Wk, Wv))
    bq, bk, bv = (np.asarray(x, dtype=np.float32) for x in (bq, bk, bv))

    Wq = Wq * SCALE
    bq = bq * SCALE
    qt = np.ascontiguousarray(query.transpose(2, 1, 0))  # [E, B, T]
    # causal masks for the 4 diagonal-crossing tile offsets, from attn_mask:
    # cmh[d][p, f] = attn_mask[f, 128 d + p], f in [0, 512)
    cmh = np.stack(
        [
            np.ascontiguousarray(attn_mask[:512, 128 * d:128 * (d + 1)].T)
            for d in range(4)
        ]
    )
    cmh = np.maximum(cmh, NEG)  # -inf -> -1e30 (exp underflows to exactly 0)
    padf = np.where(kpm, NEG, 0.0).astype(np.float32)  # [B, T]
    padh = np.ascontiguousarray(
        padf.reshape(B, 8, 128).transpose(2, 0, 1).reshape(128, B * 8)
    )

    in_maps = []
    for i in range(NCORES):
        rows = slice(i * DS, (i + 1) * DS)
        in_maps.append(
            {
                "qt": qt,
                "wq": np.ascontiguousarray(Wq[rows].T),
                "wk": np.ascontiguousarray(Wk[rows].T),
                "wv": np.ascontiguousarray(Wv[rows].T),
                "bq": np.ascontiguousarray(bq[rows, None]),
                "bk": np.ascontiguousarray(bk[rows, None]),
                "bv": np.ascontiguousarray(bv[rows, None]),
                "cm": cmh,
                "pad": padh,
                "ones": np.ones((128, 16), dtype=np.float32),
            }
        )
    return in_maps


def kernel(query, key, key_padding_mask, attn_mask, Wq, bq, Wk, bk, Wv, bv,
           num_heads):
    from concourse.bass_utils import run_bass_kernel_spmd

    assert int(num_heads) == H
    nc = _get_program()
    in_maps = _prepare_in_maps(query, key_padding_mask, attn_mask, Wq, bq, Wk,
                               bk, Wv, bv)
    res = run_bass_kernel_spmd(nc, in_maps, core_ids=list(range(NCORES)))
    full = np.concatenate(
        [res.results[i]["out"] for i in range(NCORES)], axis=2
    ).transpose(1, 0, 2)
    return np.ascontiguousarray(full)


# revision 29
# speedup vs baseline: 1.3154x; 1.0347x over previous
"""Multihead self-attention (T=1024, B=4, E=1024, H=16) on 8 TRN2 NeuronCores.

Sharding: head-parallel. Core i owns heads {2i, 2i+1} == E-rows [128i, 128i+128)
of Wq/Wk/Wv, and all 4 batches. No cross-core communication.

Per-core dataflow (all "transposed" layouts, d on partitions):
  qT/kT/vT [128, B*T] = W_slice @ query.T   (PE, float32r, K=E in 8 chunks)
  per (b, head, t-chunk of 512):
    scoresT [s=128, t<=512] = kT_tile.T-free matmul; causal tiles above the
    diagonal are skipped entirely, diagonal tiles are column-sliced.
    probs = Exp(scoresT + causal_mask + padding_bias)  (ACT, padding as
    per-partition bias; both heads share one [128, 1024] ACT op)
    outT[65, 512] += va_tile[128, 65].T @ probs  where va has a ones column,
    so row 64 accumulates the softmax denominator.
  normalize: DMA-broadcast denominator row, DVE reciprocal + multiply.
Host gathers [128, B, T] per-core outputs -> [T, B, E].
"""

import numpy as np

T, B, E, H = 1024, 4, 1024, 16
D = 64  # head dim
NCORES = 8
HPC = H // NCORES  # heads per core = 2
DS = HPC * D  # per-core E-slice = 128
TB = T * B
NEG = -1.0e30
SCALE = D**-0.5

_COMPILED = {}


def _build_program():
    import concourse.bacc as bacc
    import concourse.mybir as mybir
    import concourse.tile as tile
    from concourse.masks import make_identity

    f32 = mybir.dt.float32
    f32r = mybir.dt.float32r
    AF = mybir.ActivationFunctionType
    ALU = mybir.AluOpType

    nc = bacc.Bacc("TRN2", target_bir_lowering=False, debug=False,
                   num_devices=NCORES)

    qt = nc.dram_tensor("qt", [E, B, T], f32r, kind="ExternalInput").ap()
    wq = nc.dram_tensor("wq", [E, DS], f32r, kind="ExternalInput").ap()
    wk = nc.dram_tensor("wk", [E, DS], f32r, kind="ExternalInput").ap()
    wv = nc.dram_tensor("wv", [E, DS], f32r, kind="ExternalInput").ap()
    bq = nc.dram_tensor("bq", [DS, 1], f32, kind="ExternalInput").ap()
    bk = nc.dram_tensor("bk", [DS, 1], f32, kind="ExternalInput").ap()
    bv = nc.dram_tensor("bv", [DS, 1], f32, kind="ExternalInput").ap()
    cm = nc.dram_tensor("cm", [4, 128, 512], f32, kind="ExternalInput").ap()
    pad = nc.dram_tensor("pad", [128, B * 8], f32, kind="ExternalInput").ap()
    onesd = nc.dram_tensor("ones", [128, 16], f32r, kind="ExternalInput").ap()
    out = nc.dram_tensor("out", [B, T, DS], f32, kind="ExternalOutput").ap()

    NJ = TB // 512  # 8 token chunks of 512; chunk j covers (b=j//2, half=j%2)

    with tile.TileContext(nc) as tc:
        with (
            tc.tile_pool(name="consts", bufs=1) as consts,
            tc.tile_pool(name="qkv", bufs=NJ) as qkv,
            tc.tile_pool(name="va", bufs=B) as vap,
        ):
            w_sb = {}
            b_sb = {}
            wdrs = {"q": wq, "k": wk, "v": wv}
            for nm in ("q", "k", "v"):
                w_sb[nm] = consts.tile([128, 8, DS], f32r, name=f"w{nm}s")
            ident = consts.tile([128, 128], f32, name="ident")
            make_identity(nc, ident[:])
            cm_sb = consts.tile([128, 4, 512], f32, name="cms")
            pad_sb = consts.tile([128, B * 8], f32, name="pads")

            # ---- Phase 1: QKV projection into transposed layout ----
            qkv_t = {"q": [], "k": [], "v": []}
            ph1 = tc.tile_pool(name="rhs", bufs=3)
            rhsp = ph1.__enter__()
            ph1b = tc.tile_pool(name="psA", bufs=2, space="PSUM")
            psA = ph1b.__enter__()
            for j in range(NJ):
                qu = rhsp.tile([128, 8, 512], f32r, tag="qu", name=f"qu{j}")
                nc.sync.dma_start(
                    qu[:],
                    qt.rearrange("(c p) b t -> p c (b t)", p=128)[
                        :, :, j * 512:(j + 1) * 512
                    ],
                )
                if j == 0:
                    # const loads ride the sync queue after the first chunk
                    for nm in ("q", "k", "v"):
                        nc.sync.dma_start(
                            w_sb[nm][:],
                            wdrs[nm].rearrange("(c p) m -> p c m", p=128),
                        )
                    for nm, bdr in (("q", bq), ("k", bk), ("v", bv)):
                        bt = consts.tile([DS, 1], f32, name=f"b{nm}s")
                        nc.sync.dma_start(bt[:], bdr)
                        b_sb[nm] = bt
                elif j == 1:
                    nc.sync.dma_start(cm_sb[:], cm.rearrange("d p f -> p d f"))
                    nc.sync.dma_start(pad_sb[:], pad)
                for nm in ("q", "k", "v"):
                    ps = psA.tile([128, 512], f32, tag="proj", name=f"ps{nm}{j}")
                    for e in range(8):
                        nc.tensor.matmul(
                            ps[:],
                            w_sb[nm][:, e, :],
                            qu[:, e, :],
                            start=(e == 0),
                            stop=(e == 7),
                        )
                    dst = qkv.tile([128, 512], f32r if nm != "v" else f32, tag=f"{nm}t", name=f"{nm}t{j}")
                    nc.vector.tensor_scalar(
                        dst[:], ps[:], b_sb[nm][:], None, op0=ALU.add,
                    )
                    qkv_t[nm].append(dst)

            ph1b.__exit__(None, None, None)
            ph1.__exit__(None, None, None)

            # ---- Phase 1.5: transpose vT -> va ([s, d] with ones column) ----
            va_t = []
            for b in range(B):
                va = vap.tile([128, 8, 2, 65], f32r, tag="va", name=f"va{b}")
                nc.sync.dma_start(
                    va[:, :, :, 64:65],
                    onesd.rearrange("p (a b c) -> p a b c", a=8, b=2, c=1),
                )
                va_t.append(va)
            ph15 = tc.tile_pool(name="psTr", bufs=2, space="PSUM")
            psTr = ph15.__enter__()
            for b in range(B):
                for p in range(8):
                    j = 2 * b + p // 4
                    off = (p % 4) * 128
                    tp = psTr.tile([128, 128], f32, tag="tr", name=f"tr{b}_{p}")
                    nc.tensor.transpose(
                        tp[:], qkv_t["v"][j][:, off:off + 128], ident[:]
                    )
                    nc.vector.tensor_copy(
                        va_t[b][:, p, :, 0:64],
                        tp[:].rearrange("p (two sub) -> p two sub", two=2),
                    )
            ph15.__exit__(None, None, None)

            # ---- Phase 2: attention, software-pipelined at chunk level:
            # scores+exp of chunk k run on PE/ACT while AV matmuls of chunk
            # k-1 (whose probs are already in SBUF) fill the PE's stall slots.
            ph2 = tc.tile_pool(name="psS", bufs=2, space="PSUM")
            psS = ph2.__enter__()
            ph2b = tc.tile_pool(name="psO", bufs=4, space="PSUM")
            psO = ph2b.__enter__()
            ph2p = tc.tile_pool(name="probs", bufs=14)
            probsp = ph2p.__enter__()
            ph2o = tc.tile_pool(name="outsb", bufs=4)
            outsb = ph2o.__enter__()
            ph2q = tc.tile_pool(name="bcast", bufs=4)
            bcastp = ph2q.__enter__()

            po = {}

            def w0_of(c, p):
                dlt = p - 4 * c
                return (min(128 * dlt, 256) if dlt > 0 else 0), dlt

            def emit_scores_exp(b, c, p):
                w0, dlt = w0_of(c, p)
                j = 2 * b + c
                jk = 2 * b + p // 4
                offk = (p % 4) * 128
                ss = psS.tile([128, 2, 512], f32, tag="sc",
                              name=f"sc{b}_{c}_{p}")
                for hl in range(HPC):
                    nc.tensor.matmul(
                        ss[:, hl, w0:512],
                        qkv_t["k"][jk][hl * 64:(hl + 1) * 64,
                                       offk:offk + 128],
                        qkv_t["q"][j][hl * 64:(hl + 1) * 64, w0:512],
                        start=True,
                        stop=True,
                    )
                if dlt >= 0:
                    nc.vector.tensor_tensor(
                        ss[:, :, w0:512],
                        ss[:, :, w0:512],
                        cm_sb[:, dlt, None, w0:512].to_broadcast(
                            (128, 2, 512 - w0)),
                        ALU.add,
                    )
                pr = probsp.tile([128, 2, 512], f32r, tag="pr",
                                 name=f"pr{b}_{c}_{p}")
                nc.scalar.activation(
                    pr[:, :, w0:512],
                    ss[:, :, w0:512],
                    AF.Exp,
                    bias=pad_sb[:, b * 8 + p:b * 8 + p + 1],
                    scale=1.0,
                )
                return pr

            def emit_av(b, c, p, pr):
                w0, _ = w0_of(c, p)
                ntile = 4 * (c + 1)
                for hl in range(HPC):
                    nc.tensor.matmul(
                        po[(b, c)][hl][:, w0:512],
                        va_t[b][:, p, hl, :],
                        pr[:, hl, w0:512],
                        start=(p == 0),
                        stop=(p == ntile - 1),
                    )

            def emit_epilogue(b, c):
                for hl in range(HPC):
                    pos = bcastp.tile([65, 512], f32, tag="pos",
                                      name=f"pos{b}_{c}_{hl}")
                    nc.vector.tensor_copy(pos[:], po[(b, c)][hl][:])
                    te = psO.tile([128, 4, 65], f32, tag="po",
                                  name=f"te{b}_{c}_{hl}")
                    for g in range(4):
                        nc.tensor.transpose(
                            te[:, g, :],
                            pos[:, g * 128:(g + 1) * 128],
                            ident[0:65, 0:65],
                        )
                    rcp = outsb.tile([128, 4, 1], f32, tag="rcp",
                                     name=f"rcp{b}_{c}_{hl}")
                    nc.vector.reciprocal(rcp[:], te[:, :, 64:65])
                    of = outsb.tile([128, 4, 64], f32, tag="of",
                                    name=f"of{b}_{c}_{hl}")
                    nc.vector.tensor_tensor(
                        of[:], te[:, :, 0:64],
                        rcp[:].to_broadcast((128, 4, 64)), ALU.mult,
                    )
                    nc.gpsimd.dma_start(
                        out[b, c * 512:(c + 1) * 512,
                            hl * 64:(hl + 1) * 64].rearrange(
                                "(g tp) m -> tp g m", tp=128),
                        of[:],
                    )

            chunks = [(b, c) for b in range(B) for c in range(2)]
            prev = None
            prev_pending = []
            for (b, c) in chunks:
                ntile = 4 * (c + 1)
                po[(b, c)] = [
                    psO.tile([65, 512], f32, tag="po", name=f"po{b}_{c}_{hl}")
                    for hl in range(HPC)
                ]
                prs = []
                for p in range(ntile):
                    pr = emit_scores_exp(b, c, p)
                    if prev_pending:
                        emit_av(*prev_pending.pop(0))
                    if p == 2 and prev is not None:
                        while prev_pending:
                            emit_av(*prev_pending.pop(0))
                        emit_epilogue(*prev)
                        prev = None
                    prs.append((b, c, p, pr))
                while prev_pending:
                    emit_av(*prev_pending.pop(0))
                if prev is not None:
                    emit_epilogue(*prev)
                prev = (b, c)
                prev_pending = prs
            while prev_pending:
                emit_av(*prev_pending.pop(0))
            emit_epilogue(*prev)

            ph2q.__exit__(None, None, None)
            ph2o.__exit__(None, None, None)
            ph2p.__exit__(None, None, None)
            ph2b.__exit__(None, None, None)
            ph2.__exit__(None, None, None)

    nc.compile()
    return nc


def _get_program():
    if "nc" not in _COMPILED:
        _COMPILED["nc"] = _build_program()
    return _COMPILED["nc"]


def _prepare_in_maps(query, key_padding_mask, attn_mask, Wq, bq, Wk, bk, Wv,
                     bv):
    query = np.asarray(query, dtype=np.float32)
    attn_mask = np.asarray(attn_mask, dtype=np.float32)
    kpm = np.asarray(key_padding_mask)
    Wq, Wk, Wv = (np.asarray(w, dtype=np.float32) for w in (Wq, Wk, Wv))
    bq, bk, bv = (np.asarray(x, dtype=np.float32) for x in (bq, bk, bv))

    Wq = Wq * SCALE
    bq = bq * SCALE
    qt = np.ascontiguousarray(query.transpose(2, 1, 0))  # [E, B, T]
    # causal masks for the 4 diagonal-crossing tile offsets, from attn_mask:
    # cmh[d][p, f] = attn_mask[f, 128 d + p], f in [0, 512)
    cmh = np.stack(
        [
            np.ascontiguousarray(attn_mask[:512, 128 * d:128 * (d + 1)].T)
            for d in range(4)
        ]
    )
    cmh = np.maximum(cmh, NEG)  # -inf -> -1e30 (exp underflows to exactly 0)
    padf = np.where(kpm, NEG, 0.0).astype(np.float32)  # [B, T]
    padh = np.ascontiguousarray(
        padf.reshape(B, 8, 128).transpose(2, 0, 1).reshape(128, B * 8)
    )

    in_maps = []
    for i in range(NCORES):
        rows = slice(i * DS, (i + 1) * DS)
        in_maps.append(
            {
                "qt": qt,
                "wq": np.ascontiguousarray(Wq[rows].T),
                "wk": np.ascontiguousarray(Wk[rows].T),
                "wv": np.ascontiguousarray(Wv[rows].T),
                "bq": np.ascontiguousarray(bq[rows, None]),
                "bk": np.ascontiguousarray(bk[rows, None]),
                "bv": np.ascontiguousarray(bv[rows, None]),
                "cm": cmh,
                "pad": padh,
                "ones": np.ones((128, 16), dtype=np.float32),
            }
        )
    return in_maps


def kernel(query, key, key_padding_mask, attn_mask, Wq, bq, Wk, bk, Wv, bv,
           num_heads):
    from concourse.bass_utils import run_bass_kernel_spmd

    assert int(num_heads) == H
    nc = _get_program()
    in_maps = _prepare_in_maps(query, key_padding_mask, attn_mask, Wq, bq, Wk,
                               bk, Wv, bv)
    res = run_bass_kernel_spmd(nc, in_maps, core_ids=list(range(NCORES)))
    full = np.concatenate(
        [res.results[i]["out"] for i in range(NCORES)], axis=2
    ).transpose(1, 0, 2)
    return np.ascontiguousarray(full)


# revision 31
# speedup vs baseline: 1.3334x; 1.0137x over previous
"""Multihead self-attention (T=1024, B=4, E=1024, H=16) on 8 TRN2 NeuronCores.

Sharding: head-parallel. Core i owns heads {2i, 2i+1} == E-rows [128i, 128i+128)
of Wq/Wk/Wv, and all 4 batches. No cross-core communication.

Per-core dataflow (all "transposed" layouts, d on partitions):
  qT/kT/vT [128, B*T] = W_slice @ query.T   (PE, float32r, K=E in 8 chunks)
  per (b, head, t-chunk of 512):
    scoresT [s=128, t<=512] = kT_tile.T-free matmul; causal tiles above the
    diagonal are skipped entirely, diagonal tiles are column-sliced.
    probs = Exp(scoresT + causal_mask + padding_bias)  (ACT, padding as
    per-partition bias; both heads share one [128, 1024] ACT op)
    outT[65, 512] += va_tile[128, 65].T @ probs  where va has a ones column,
    so row 64 accumulates the softmax denominator.
  normalize: DMA-broadcast denominator row, DVE reciprocal + multiply.
Host gathers [128, B, T] per-core outputs -> [T, B, E].
"""

import numpy as np

T, B, E, H = 1024, 4, 1024, 16
D = 64  # head dim
NCORES = 8
HPC = H // NCORES  # heads per core = 2
DS = HPC * D  # per-core E-slice = 128
TB = T * B
NEG = -1.0e30
SCALE = D**-0.5

_COMPILED = {}


def _build_program():
    import concourse.bacc as bacc
    import concourse.mybir as mybir
    import concourse.tile as tile
    from concourse.masks import make_identity

    f32 = mybir.dt.float32
    f32r = mybir.dt.float32r
    AF = mybir.ActivationFunctionType
    ALU = mybir.AluOpType

    nc = bacc.Bacc("TRN2", target_bir_lowering=False, debug=False,
                   num_devices=NCORES)

    qt = nc.dram_tensor("qt", [E, B, T], f32r, kind="ExternalInput").ap()
    wq = nc.dram_tensor("wq", [E, DS], f32r, kind="ExternalInput").ap()
    wk = nc.dram_tensor("wk", [E, DS], f32r, kind="ExternalInput").ap()
    wv = nc.dram_tensor("wv", [E, DS], f32r, kind="ExternalInput").ap()
    bq = nc.dram_tensor("bq", [DS, 1], f32, kind="ExternalInput").ap()
    bk = nc.dram_tensor("bk", [DS, 1], f32, kind="ExternalInput").ap()
    bv = nc.dram_tensor("bv", [DS, 1], f32, kind="ExternalInput").ap()
    cm = nc.dram_tensor("cm", [4, 128, 512], f32, kind="ExternalInput").ap()
    pad = nc.dram_tensor("pad", [128, B * 8], f32, kind="ExternalInput").ap()
    onesd = nc.dram_tensor("ones", [128, 16], f32r, kind="ExternalInput").ap()
    out = nc.dram_tensor("out", [B, T, DS], f32, kind="ExternalOutput").ap()

    NJ = TB // 512  # 8 token chunks of 512; chunk j covers (b=j//2, half=j%2)

    with tile.TileContext(nc) as tc:
        with (
            tc.tile_pool(name="consts", bufs=1) as consts,
            tc.tile_pool(name="qkv", bufs=NJ) as qkv,
            tc.tile_pool(name="va", bufs=B) as vap,
        ):
            w_sb = {}
            b_sb = {}
            wdrs = {"q": wq, "k": wk, "v": wv}
            for nm in ("q", "k", "v"):
                w_sb[nm] = consts.tile([128, 8, DS], f32r, name=f"w{nm}s")
            ident = consts.tile([128, 128], f32, name="ident")
            make_identity(nc, ident[:])
            cm_sb = consts.tile([128, 4, 512], f32, name="cms")
            pad_sb = consts.tile([128, B * 8], f32, name="pads")

            # ---- Phase 1: QKV projection into transposed layout ----
            qkv_t = {"q": [], "k": [], "v": []}
            ph1 = tc.tile_pool(name="rhs", bufs=3)
            rhsp = ph1.__enter__()
            ph1b = tc.tile_pool(name="psA", bufs=2, space="PSUM")
            psA = ph1b.__enter__()
            ph15 = tc.tile_pool(name="psTr", bufs=2, space="PSUM")
            psTr = ph15.__enter__()
            va_t = []
            for b in range(B):
                va = vap.tile([128, 8, 2, 65], f32r, tag="va", name=f"va{b}")
                nc.sync.dma_start(
                    va[:, :, :, 64:65],
                    onesd.rearrange("p (a b c) -> p a b c", a=8, b=2, c=1),
                )
                va_t.append(va)
            for j in range(NJ):
                qu = rhsp.tile([128, 8, 512], f32r, tag="qu", name=f"qu{j}")
                nc.sync.dma_start(
                    qu[:],
                    qt.rearrange("(c p) b t -> p c (b t)", p=128)[
                        :, :, j * 512:(j + 1) * 512
                    ],
                )
                if j == 0:
                    # const loads ride the sync queue after the first chunk
                    for nm in ("q", "k", "v"):
                        nc.sync.dma_start(
                            w_sb[nm][:],
                            wdrs[nm].rearrange("(c p) m -> p c m", p=128),
                        )
                    for nm, bdr in (("q", bq), ("k", bk), ("v", bv)):
                        bt = consts.tile([DS, 1], f32, name=f"b{nm}s")
                        nc.sync.dma_start(bt[:], bdr)
                        b_sb[nm] = bt
                elif j == 1:
                    nc.sync.dma_start(cm_sb[:], cm.rearrange("d p f -> p d f"))
                    nc.sync.dma_start(pad_sb[:], pad)
                for nm in ("q", "k", "v"):
                    ps = psA.tile([128, 512], f32, tag="proj", name=f"ps{nm}{j}")
                    for e in range(8):
                        nc.tensor.matmul(
                            ps[:],
                            w_sb[nm][:, e, :],
                            qu[:, e, :],
                            start=(e == 0),
                            stop=(e == 7),
                        )
                    dst = qkv.tile([128, 512], f32r if nm != "v" else f32, tag=f"{nm}t", name=f"{nm}t{j}")
                    nc.vector.tensor_scalar(
                        dst[:], ps[:], b_sb[nm][:], None, op0=ALU.add,
                    )
                    qkv_t[nm].append(dst)
                if j % 2 == 1:
                    b = j // 2
                    for p in range(8):
                        jj = 2 * b + p // 4
                        off = (p % 4) * 128
                        tp = psTr.tile([128, 128], f32, tag="tr",
                                       name=f"tr{b}_{p}")
                        nc.tensor.transpose(
                            tp[:], qkv_t["v"][jj][:, off:off + 128], ident[:]
                        )
                        nc.vector.tensor_copy(
                            va_t[b][:, p, :, 0:64],
                            tp[:].rearrange("p (two sub) -> p two sub", two=2),
                        )

            ph15.__exit__(None, None, None)
            ph1b.__exit__(None, None, None)
            ph1.__exit__(None, None, None)

            # ---- Phase 2: attention, software-pipelined at chunk level:
            # scores+exp of chunk k run on PE/ACT while AV matmuls of chunk
            # k-1 (whose probs are already in SBUF) fill the PE's stall slots.
            ph2 = tc.tile_pool(name="psS", bufs=2, space="PSUM")
            psS = ph2.__enter__()
            ph2b = tc.tile_pool(name="psO", bufs=4, space="PSUM")
            psO = ph2b.__enter__()
            ph2p = tc.tile_pool(name="probs", bufs=14)
            probsp = ph2p.__enter__()
            ph2o = tc.tile_pool(name="outsb", bufs=4)
            outsb = ph2o.__enter__()
            ph2q = tc.tile_pool(name="bcast", bufs=4)
            bcastp = ph2q.__enter__()

            po = {}

            def w0_of(c, p):
                dlt = p - 4 * c
                return (min(128 * dlt, 256) if dlt > 0 else 0), dlt

            def emit_scores_exp(b, c, p):
                w0, dlt = w0_of(c, p)
                j = 2 * b + c
                jk = 2 * b + p // 4
                offk = (p % 4) * 128
                ss = psS.tile([128, 2, 512], f32, tag="sc",
                              name=f"sc{b}_{c}_{p}")
                for hl in range(HPC):
                    nc.tensor.matmul(
                        ss[:, hl, w0:512],
                        qkv_t["k"][jk][hl * 64:(hl + 1) * 64,
                                       offk:offk + 128],
                        qkv_t["q"][j][hl * 64:(hl + 1) * 64, w0:512],
                        start=True,
                        stop=True,
                    )
                if dlt >= 0:
                    nc.vector.tensor_tensor(
                        ss[:, :, w0:512],
                        ss[:, :, w0:512],
                        cm_sb[:, dlt, None, w0:512].to_broadcast(
                            (128, 2, 512 - w0)),
                        ALU.add,
                    )
                pr = probsp.tile([128, 2, 512], f32r, tag="pr",
                                 name=f"pr{b}_{c}_{p}")
                nc.scalar.activation(
                    pr[:, :, w0:512],
                    ss[:, :, w0:512],
                    AF.Exp,
                    bias=pad_sb[:, b * 8 + p:b * 8 + p + 1],
                    scale=1.0,
                )
                return pr

            def emit_av(b, c, p, pr):
                w0, _ = w0_of(c, p)
                ntile = 4 * (c + 1)
                for hl in range(HPC):
                    nc.tensor.matmul(
                        po[(b, c)][hl][:, w0:512],
                        va_t[b][:, p, hl, :],
                        pr[:, hl, w0:512],
                        start=(p == 0),
                        stop=(p == ntile - 1),
                    )

            def emit_epilogue(b, c):
                for hl in range(HPC):
                    pos = bcastp.tile([65, 512], f32, tag="pos",
                                      name=f"pos{b}_{c}_{hl}")
                    nc.vector.tensor_copy(pos[:], po[(b, c)][hl][:])
                    te = psO.tile([128, 4, 65], f32, tag="po",
                                  name=f"te{b}_{c}_{hl}")
                    for g in range(4):
                        nc.tensor.transpose(
                            te[:, g, :],
                            pos[:, g * 128:(g + 1) * 128],
                            ident[0:65, 0:65],
                        )
                    rcp = outsb.tile([128, 4, 1], f32, tag="rcp",
                                     name=f"rcp{b}_{c}_{hl}")
                    nc.vector.reciprocal(rcp[:], te[:, :, 64:65])
                    of = outsb.tile([128, 4, 64], f32, tag="of",
                                    name=f"of{b}_{c}_{hl}")
                    nc.vector.tensor_tensor(
                        of[:], te[:, :, 0:64],
                        rcp[:].to_broadcast((128, 4, 64)), ALU.mult,
                    )
                    nc.gpsimd.dma_start(
                        out[b, c * 512:(c + 1) * 512,
                            hl * 64:(hl + 1) * 64].rearrange(
                                "(g tp) m -> tp g m", tp=128),
                        of[:],
                    )

            chunks = [(b, c) for b in range(B) for c in range(2)]
            prev = None
            prev_pending = []
            for (b, c) in chunks:
                ntile = 4 * (c + 1)
                po[(b, c)] = [
                    psO.tile([65, 512], f32, tag="po", name=f"po{b}_{c}_{hl}")
                    for hl in range(HPC)
                ]
                prs = []
                for p in range(ntile):
                    pr = emit_scores_exp(b, c, p)
                    if prev_pending:
                        emit_av(*prev_pending.pop(0))
                    if p == 2 and prev is not None:
                        while prev_pending:
                            emit_av(*prev_pending.pop(0))
                        emit_epilogue(*prev)
                        prev = None
                    prs.append((b, c, p, pr))
                while prev_pending:
                    emit_av(*prev_pending.pop(0))
                if prev is not None:
                    emit_epilogue(*prev)
                prev = (b, c)
                prev_pending = prs
            while prev_pending:
                emit_av(*prev_pending.pop(0))
            emit_epilogue(*prev)

            ph2q.__exit__(None, None, None)
            ph2o.__exit__(None, None, None)
            ph2p.__exit__(None, None, None)
            ph2b.__exit__(None, None, None)
            ph2.__exit__(None, None, None)

    nc.compile()
    return nc


def _get_program():
    if "nc" not in _COMPILED:
        _COMPILED["nc"] = _build_program()
    return _COMPILED["nc"]


def _prepare_in_maps(query, key_padding_mask, attn_mask, Wq, bq, Wk, bk, Wv,
                     bv):
    query = np.asarray(query, dtype=np.float32)
    attn_mask = np.asarray(attn_mask, dtype=np.float32)
    kpm = np.asarray(key_padding_mask)
    Wq, Wk, Wv = (np.asarray(w, dtype=np.float32) for w in (Wq, Wk, Wv))
    bq, bk, bv = (np.asarray(x, dtype=np.float32) for x in (bq, bk, bv))

    Wq = Wq * SCALE
    bq = bq * SCALE
    qt = np.ascontiguousarray(query.transpose(2, 1, 0))  # [E, B, T]
    # causal masks for the 4 diagonal-crossing tile offsets, from attn_mask:
    # cmh[d][p, f] = attn_mask[f, 128 d + p], f in [0, 512)
    cmh = np.stack(
        [
            np.ascontiguousarray(attn_mask[:512, 128 * d:128 * (d + 1)].T)
            for d in range(4)
        ]
    )
    cmh = np.maximum(cmh, NEG)  # -inf -> -1e30 (exp underflows to exactly 0)
    padf = np.where(kpm, NEG, 0.0).astype(np.float32)  # [B, T]
    padh = np.ascontiguousarray(
        padf.reshape(B, 8, 128).transpose(2, 0, 1).reshape(128, B * 8)
    )

    in_maps = []
    for i in range(NCORES):
        rows = slice(i * DS, (i + 1) * DS)
        in_maps.append(
            {
                "qt": qt,
                "wq": np.ascontiguousarray(Wq[rows].T),
                "wk": np.ascontiguousarray(Wk[rows].T),
                "wv": np.ascontiguousarray(Wv[rows].T),
                "bq": np.ascontiguousarray(bq[rows, None]),
                "bk": np.ascontiguousarray(bk[rows, None]),
                "bv": np.ascontiguousarray(bv[rows, None]),
                "cm": cmh,
                "pad": padh,
                "ones": np.ones((128, 16), dtype=np.float32),
            }
        )
    return in_maps


def kernel(query, key, key_padding_mask, attn_mask, Wq, bq, Wk, bk, Wv, bv,
           num_heads):
    from concourse.bass_utils import run_bass_kernel_spmd

    assert int(num_heads) == H
    nc = _get_program()
    in_maps = _prepare_in_maps(query, key_padding_mask, attn_mask, Wq, bq, Wk,
                               bk, Wv, bv)
    res = run_bass_kernel_spmd(nc, in_maps, core_ids=list(range(NCORES)))
    full = np.concatenate(
        [res.results[i]["out"] for i in range(NCORES)], axis=2
    ).transpose(1, 0, 2)
    return np.ascontiguousarray(full)
